# revision 52
# baseline (speedup 1.0000x reference)
"""Trainium2 Bass kernel for BitNet multi-head attention (nn_MultiHeadAttention_62294205661880).

Sharding: 8 cores = 2 batches x 4 head-groups (4 heads each).  Each core
computes qkv projection, RoPE, causal attention and a column-parallel slice
of the output projection for its (batch, head-group); the host sums the 4
partial out-projections per batch.

BitNet quantization is folded on the host: weights are uploaded as exact
ternary {-1,0,+1} fp8 matrices; scale_qkv^2/sqrt(dh) is folded into the
softmax exp() scale and scale_qkv*scale_out into a final host-side scalar.

FP8 acceleration (per-stage flags in CFG).  fp8 DoubleRow matmuls contract
two 128-row k-tiles per pass (2x flops/cycle vs bf16); measured error budget
(rel err vs 2e-2 gate): x->fp8 noise on the q/k path costs ~0.007 rel, on
the v path ~0.017 (blocked), so:
  a8   - q/k projections from 1-stream fp8 x (fast, small scores-path noise)
  v2s  - v projection from x8 + fp8-residual 2-stream (bf16-accurate, bf16
         speed, but keeps the whole phase on the fp8 DoubleRow pipeline)
  ones8- softmax denominator: gpsimd copies pp to fp8, the all-ones
         stationary matmul runs DoubleRow over ki-block pairs (denominator
         averages ~128+ keys so pp quantization noise is negligible there)
  o8   - (optional nibble) out-projection aoT in fp8 -- costs ~0.017 rel,
         disabled by default

Device layout trick: everything is computed transposed.  Q_T/K_T come out of
the projection as [dh, S]; scores are computed as s_T[k, q]; the softmax
denominator sums over the partition (key) dim via an all-ones stationary
matmul; AV produces out_T[dh, q] which feeds the output projection directly.
Softmax skips the max-subtraction: scores are bounded (~+-2) because the
BitNet weight scale is tiny, so exp() is safe -- this also makes it safe to
run exp over never-written PSUM regions (any stale f32 times the ~2e-5 exp
scale stays finite), which lets one ACT instruction cover a pair of
diagonal score blocks whose valid column ranges differ.
"""

import sys
import types

import numpy as np
import ml_dtypes

import concourse.bass as bass
import concourse.mybir as mybir
import concourse.tile as tile
from concourse import bacc
from concourse.bass_utils import run_bass_kernel_spmd

D_MODEL = 2048
N_HEADS = 16
D_HEAD = 128
SEQ = 2048
BATCH = 2
ROPE_BASE = 10000.0

N_CORES = 8
HPC = 4  # heads per core
R_LOCAL = HPC * D_HEAD  # 512 local q (or k, or v) rows per core
MO = D_MODEL // 128  # 16 contraction blocks
NKI = SEQ // 128  # 16 key blocks
NQC = SEQ // 512  # 4 query chunks of 512
NSB = SEQ // 128  # 16 seq blocks (v / proj)

BF16 = mybir.dt.bfloat16
F32 = mybir.dt.float32
NPBF16 = ml_dtypes.bfloat16
NPFP8 = ml_dtypes.float8_e4m3
FP8 = mybir.dt.float8e4
DR = mybir.MatmulPerfMode.DoubleRow

# o8h: out-projection aoT in fp8 for heads 0-1 only (one DoubleRow pair),
#      heads 2-3 bf16 — half the aoT quantization noise of full o8
CFG = dict(a8=True, v2s=True, ones8=False, o8=False, o8h=True, rope4=False)

LAST_RESULT = None  # BassKernelResults of the most recent run (for test.py)
_PROG_CACHE = {}
PROFILE = False  # test.py sets True to capture an NTFF profile / HW exec time


def _enable_profiling() -> bool:
    """Install the axon NTFF profile hook glue if the image lacks
    ``antenv.axon_hooks`` (boot degrades silently without it), and skip
    the artifact upload (no bucket access in this container)."""
    try:
        from antenv.axon_hooks import get_axon_ntff_profile_hook  # noqa: F401

        ok = get_axon_ntff_profile_hook() is not None
    except ImportError:
        ok = False
        import antenv

        mod = types.ModuleType("antenv.axon_hooks")
        mod._hook = None
        mod.set_axon_ntff_profile_hook = lambda h: setattr(mod, "_hook", h)
        mod.get_axon_ntff_profile_hook = lambda: mod._hook
        sys.modules["antenv.axon_hooks"] = mod
        antenv.axon_hooks = mod
        try:
            from trn_agent_boot.trn_boot import _ntff_profile_via_ctypes

            hook = _ntff_profile_via_ctypes("/opt/axon/libaxon_pjrt.so")
            if hook is not None:
                mod._hook = hook
                ok = True
        except Exception as e:  # profiling is best-effort
            print(f"ntff profile hook install failed: {e}", file=sys.stderr)
    if ok:
        import concourse.bass_utils as _bu

        _bu.upload_artifacts = lambda tmpdir: tmpdir
    return ok


def _build_program(causal: bool, exp_scale: float, cfg: dict) -> bass.Bass:
    a8 = cfg["a8"]
    v2s = cfg["v2s"] and a8
    ones8 = cfg["ones8"] and causal
    o8 = cfg["o8"] and causal
    o8h = cfg["o8h"] and causal and not o8

    nc = bacc.Bacc(None)
    S = SEQ

    x_dt = FP8 if a8 else BF16
    wo_dt = FP8 if o8 else BF16
    ao_dt = FP8 if o8 else BF16

    xT_d = nc.dram_tensor("xT", [D_MODEL, S], x_dt, kind="ExternalInput")
    if v2s:
        rT_d = nc.dram_tensor("rT", [D_MODEL, S], FP8, kind="ExternalInput")
    wqT_d = nc.dram_tensor("wqT", [D_MODEL, R_LOCAL], FP8, kind="ExternalInput")
    wkT_d = nc.dram_tensor("wkT", [D_MODEL, R_LOCAL], FP8, kind="ExternalInput")
    wvT_d = nc.dram_tensor("wvT", [D_MODEL, R_LOCAL], FP8, kind="ExternalInput")
    woT_d = nc.dram_tensor("woT", [R_LOCAL, D_MODEL], wo_dt, kind="ExternalInput")
    if o8h:
        woT8_d = nc.dram_tensor(
            "woT8", [2 * D_HEAD, D_MODEL], FP8, kind="ExternalInput"
        )
    # cos rows 0:64, sin rows 64:128
    cs_d = nc.dram_tensor("cossinT", [128, S], BF16, kind="ExternalInput")
    # swapped: sin rows 0:64, cos rows 64:128 (keeps TensorTensor base partitions equal)
    sc_d = nc.dram_tensor("sincosT", [128, S], BF16, kind="ExternalInput")
    if causal:
        # 16 transposed diagonal 128x128 mask blocks, side by side
        maskd_d = nc.dram_tensor("maskd", [128, S], BF16, kind="ExternalInput")
    else:
        maskf_d = nc.dram_tensor("maskf", [S, S], BF16, kind="ExternalInput")
    out_d = nc.dram_tensor("out", [S, D_MODEL], BF16, kind="ExternalOutput")

    xT_v = xT_d[:].rearrange("(mo p) s -> p mo s", p=128)
    if v2s:
        rT_v = rT_d[:].rearrange("(mo p) s -> p mo s", p=128)
    wqT_v = wqT_d[:].rearrange("(mo p) r -> p mo r", p=128)
    wkT_v = wkT_d[:].rearrange("(mo p) r -> p mo r", p=128)
    wvT_v = wvT_d[:].rearrange("(mo p) r -> p mo r", p=128)
    woT_v = woT_d[:].rearrange("(h p) o -> p h o", p=128)
    if o8h:
        woT8_v = woT8_d[:].rearrange("(h p) o -> p h o", p=128)
    if not causal:
        maskf_v = maskf_d[:].rearrange("(ko p) q -> p ko q", p=128)

    with tile.TileContext(nc) as tc:
        with tc.tile_pool(name="pers", bufs=1) as pers:
            # ---- persistent SBUF tensors (live across both phases) ----
            q_rot = pers.tile([128, HPC, S], BF16, tag="qrot")
            k_rot = pers.tile([128, HPC, S], BF16, tag="krot")
            v_sb = pers.tile([128, NKI, R_LOCAL], BF16, tag="vsb")
            aoT = pers.tile([128, HPC, S], ao_dt, tag="aoT")
            wo = pers.tile([128, HPC, D_MODEL], wo_dt, tag="wo")
            if o8h:
                aoT8 = pers.tile([128, 2, S], FP8, tag="aoT8")
                wo8 = pers.tile([128, 2, D_MODEL], FP8, tag="wo8")
            ones_t = pers.tile([128, 128], BF16, tag="ones")
            if ones8:
                ones2 = pers.tile([128, 2, 128], FP8, tag="ones2")
                nc.vector.memset(ones2[:], 1.0)
            warm = pers.tile([128, 2], BF16, tag="warm")
            if causal:
                maskd = pers.tile([128, S], BF16, tag="maskd")
            nc.vector.memset(ones_t[:, :], 1.0)
            # load the exp table set first so no ACT table switch happens
            # mid-kernel (Copy lives in every set).
            nc.vector.memset(warm[:, 0:1], 1.0)
            nc.scalar.activation(
                warm[:, 1:2], warm[:, 0:1], mybir.ActivationFunctionType.Exp
            )

            # ================= phase A: QKV projection + RoPE =================
            with (
                tc.tile_pool(name="xtp", bufs=1) as xtp,
                tc.tile_pool(name="wp", bufs=1) as wp,
                tc.tile_pool(name="raw", bufs=2) as rawp,
                tc.tile_pool(name="w8", bufs=6) as w8p,
                tc.tile_pool(name="tmp", bufs=2) as tmpp,
                tc.tile_pool(name="psA", bufs=4, space="PSUM") as psA,
            ):
                w_dt = FP8 if a8 else BF16
                xt = xtp.tile([128, MO, S], x_dt, tag="xt")
                if v2s:
                    rt = xtp.tile([128, MO, S], FP8, tag="rt")
                wq = wp.tile([128, MO, R_LOCAL], w_dt, tag="wq")
                wk = wp.tile([128, MO, R_LOCAL], w_dt, tag="wk")
                wv = wp.tile([128, MO, R_LOCAL], w_dt, tag="wv")
                cs_t = wp.tile([128, S], BF16, tag="cs")
                sc_t = wp.tile([128, S], BF16, tag="sc")

                def load_w(dst, view, mo4):
                    """load weight m-blocks mo4..mo4+3 (fp8 direct or cast)."""
                    if a8:  # straight fp8 DMA, no cast
                        nc.sync.dma_start(
                            out=dst[:, mo4 : mo4 + 4, :], in_=view[:, mo4 : mo4 + 4, :]
                        )
                    else:
                        for mo in range(mo4, mo4 + 4):
                            st = w8p.tile([128, R_LOCAL], FP8, tag="w8")
                            nc.sync.dma_start(out=st[:, :], in_=view[:, mo, :])
                            nc.vector.tensor_copy(dst[:, mo, :], st[:, :])

                # critical path first: wq chunk 0 + first x pairs feed the
                # h=0 q projection; w chunks and rope tables land ahead of
                # the bulk x blocks they gate.
                # each dma_start costs ~0.64us of serialized issue time on the
                # sync engine, so phase A uses few, chunky transfers ordered
                # by first-use: wq/wk chunk 0, then ALL x (the h=0 q/k
                # projections burn through every m-pair within ~14us), then
                # later weight chunks, rope tables, v/residual, phase-B data.
                load_w(wq, wqT_v, 0)
                nc.sync.dma_start(out=xt[:, 0:2, 0:1024], in_=xT_v[:, 0:2, 0:1024])
                load_w(wk, wkT_v, 0)
                nc.sync.dma_start(
                    out=xt[:, 0:2, 1024:2048], in_=xT_v[:, 0:2, 1024:2048]
                )
                nc.sync.dma_start(out=xt[:, 2:4, :], in_=xT_v[:, 2:4, :])
                nc.sync.dma_start(out=xt[:, 4:8, :], in_=xT_v[:, 4:8, :])
                load_w(wq, wqT_v, 4)
                load_w(wk, wkT_v, 4)
                nc.sync.dma_start(out=xt[:, 8:12, :], in_=xT_v[:, 8:12, :])
                nc.sync.dma_start(out=xt[:, 12:16, :], in_=xT_v[:, 12:16, :])
                for c4m in range(2, 4):
                    load_w(wq, wqT_v, 4 * c4m)
                    load_w(wk, wkT_v, 4 * c4m)
                nc.sync.dma_start(out=cs_t[:, :], in_=cs_d[:, :])
                nc.sync.dma_start(out=sc_t[:, :], in_=sc_d[:, :])
                for c4m in range(4):
                    load_w(wv, wvT_v, 4 * c4m)
                    if v2s:
                        nc.sync.dma_start(
                            out=rt[:, 4 * c4m : 4 * c4m + 4, :],
                            in_=rT_v[:, 4 * c4m : 4 * c4m + 4, :],
                        )
                if causal:  # not needed until phase B
                    nc.sync.dma_start(out=maskd[:, :], in_=maskd_d[:, :])
                # prefetch the out-projection weights behind everything else
                # so phase B starts without a DMA wait
                if o8h:
                    nc.sync.dma_start(out=wo8[:, :, :], in_=woT8_v[:, :, :])
                    nc.sync.dma_start(out=wo[:, 2:4, :], in_=woT_v[:, 2:4, :])
                else:
                    for oc in range(D_MODEL // 512):
                        nc.sync.dma_start(
                            out=wo[:, :, oc * 512 : (oc + 1) * 512],
                            in_=woT_v[:, :, oc * 512 : (oc + 1) * 512],
                        )

                def rope(h, raw, dst):
                    """NeoX rotary: low = t1*c - t2*s ; hi = t1*s + t2*c."""
                    d_lo = dst[0:64, h, :]
                    d_hi = dst[64:128, h, :]
                    if cfg["rope4"]:
                        # 4 full-width ops, cross-base-partition sub/add
                        p1 = tmpp.tile([128, S], BF16, tag="tmp")
                        p2 = tmpp.tile([128, S], BF16, tag="tmp")
                        nc.vector.tensor_mul(p1[:, :], raw[:, :], cs_t[:, :])
                        nc.vector.tensor_mul(p2[:, :], raw[:, :], sc_t[:, :])
                        nc.vector.tensor_sub(d_lo, p1[0:64, :], p1[64:128, :])
                        nc.vector.tensor_add(d_hi, p2[0:64, :], p2[64:128, :])
                    else:
                        ta = tmpp.tile([64, S], BF16, tag="tmp")
                        tb = tmpp.tile([64, S], BF16, tag="tmp")
                        nc.vector.tensor_mul(ta[:, :], raw[0:64, :], cs_t[0:64, :])
                        nc.vector.tensor_mul(tb[:, :], raw[64:128, :], cs_t[64:128, :])
                        nc.vector.tensor_sub(d_lo, ta[:, :], tb[:, :])
                        tc2 = tmpp.tile([64, S], BF16, tag="tmp")
                        td = tmpp.tile([64, S], BF16, tag="tmp")
                        nc.vector.tensor_mul(tc2[:, :], raw[0:64, :], sc_t[0:64, :])
                        nc.vector.tensor_mul(td[:, :], raw[64:128, :], sc_t[64:128, :])
                        nc.vector.tensor_add(d_hi, tc2[:, :], td[:, :])

                def project_qk(h):
                    # two half-width psum tiles per projection: finer WAR
                    # rotation lets the next head's matmuls start while this
                    # head's second half is still being evicted
                    psq0 = psA.tile([128, 1024], F32, tag="psA")
                    psq1 = psA.tile([128, 1024], F32, tag="psA")
                    psk0 = psA.tile([128, 1024], F32, tag="psA")
                    psk1 = psA.tile([128, 1024], F32, tag="psA")
                    hs = slice(h * 128, (h + 1) * 128)

                    def mm_into(halves, w_sb, m_sl, first, last, pm):
                        for c4 in range(4):
                            cs_ = slice(c4 * 512, (c4 + 1) * 512)
                            tgt = halves[c4 // 2][:, (c4 % 2) * 512 : (c4 % 2) * 512 + 512]
                            nc.tensor.matmul(
                                tgt, w_sb[:, m_sl, hs], xt[:, m_sl, cs_],
                                start=first, stop=last, perf_mode=pm,
                            )

                    if a8:
                        for m2 in range(MO // 2):
                            ms = slice(2 * m2, 2 * m2 + 2)
                            mm_into((psq0, psq1), wq, ms, m2 == 0, m2 == MO // 2 - 1, DR)
                            mm_into((psk0, psk1), wk, ms, m2 == 0, m2 == MO // 2 - 1, DR)
                    else:
                        for m in range(MO):
                            mm_into((psq0, psq1), wq, m, m == 0, m == MO - 1, None)
                            mm_into((psk0, psk1), wk, m, m == 0, m == MO - 1, None)
                    q_raw = rawp.tile([128, S], BF16, tag="raw")
                    nc.scalar.copy(q_raw[:, 0:1024], psq0[:, :])
                    nc.scalar.copy(q_raw[:, 1024:2048], psq1[:, :])
                    rope(h, q_raw, q_rot)
                    k_raw = rawp.tile([128, S], BF16, tag="raw")
                    nc.scalar.copy(k_raw[:, 0:1024], psk0[:, :])
                    nc.scalar.copy(k_raw[:, 1024:2048], psk1[:, :])
                    rope(h, k_raw, k_rot)

                for h in range(HPC):
                    project_qk(h)

                # ---- V projection (natural layout [s, r]) ----
                for sb2 in range(NSB // 2):
                    ps = psA.tile([128, 1024], F32, tag="psA")
                    for part in range(2):
                        sb = sb2 * 2 + part
                        ss = slice(sb * 128, (sb + 1) * 128)
                        ps_ = ps[:, part * 512 : part * 512 + 512]
                        if a8:
                            for m2 in range(MO // 2):
                                ms = slice(2 * m2, 2 * m2 + 2)
                                nc.tensor.matmul(
                                    ps_, xt[:, ms, ss], wv[:, ms, :],
                                    start=(m2 == 0),
                                    stop=(not v2s and m2 == MO // 2 - 1),
                                    perf_mode=DR,
                                )
                            if v2s:  # residual stream restores bf16 accuracy
                                for m2 in range(MO // 2):
                                    ms = slice(2 * m2, 2 * m2 + 2)
                                    nc.tensor.matmul(
                                        ps_, rt[:, ms, ss], wv[:, ms, :],
                                        start=False, stop=(m2 == MO // 2 - 1),
                                        perf_mode=DR,
                                    )
                        else:
                            for m in range(MO):
                                nc.tensor.matmul(
                                    ps_, xt[:, m, ss], wv[:, m, :],
                                    start=(m == 0), stop=(m == MO - 1),
                                )
                    dst = v_sb[:, sb2 * 2 : sb2 * 2 + 2, :]
                    nc.scalar.copy(dst, ps[:, :])

            # ================= phase B: attention + out-projection =============
            with (
                tc.tile_pool(name="pp", bufs=8) as ppp,
                tc.tile_pool(name="pp8", bufs=8) as pp8p,
                tc.tile_pool(name="rcp", bufs=3) as rcp,
                tc.tile_pool(name="osb", bufs=4) as osbp,
                tc.tile_pool(name="mblk", bufs=4) as mblkp,
                tc.tile_pool(name="sp", bufs=4, space="PSUM") as spp,
                tc.tile_pool(name="acc", bufs=2, space="PSUM") as accp,
            ):
                evict_flip = [0]
                EXPF = mybir.ActivationFunctionType.Exp

                def attn_causal(qc, h, sums, avp):
                    """per-ki pipeline; full-block denominators collected as
                    fp8 pairs and summed by deferred DoubleRow matmuls."""
                    q_lo = qc * 512
                    hs = slice(h * 128, (h + 1) * 128)
                    nki_here = 4 * qc + 4
                    pp8s = []
                    for ki in range(nki_here):
                        diag = ki >= 4 * qc
                        q0 = 128 * (ki - 4 * qc) if diag else 0
                        spbv = spp.tile([128, 512], F32, tag="sp")
                        pp = ppp.tile([128, 512], BF16, tag="pp")
                        nc.tensor.matmul(
                            spbv[:, q0:512],
                            k_rot[:, h, ki * 128 : (ki + 1) * 128],
                            q_rot[:, h, q_lo + q0 : q_lo + 512],
                            start=True, stop=True,
                        )
                        nc.scalar.activation(
                            pp[:, q0:512], spbv[:, q0:512], EXPF,
                            scale=float(exp_scale),
                        )
                        if diag:
                            nc.vector.tensor_mul(
                                pp[:, q0 : q0 + 128], pp[:, q0 : q0 + 128],
                                maskd[:, ki * 128 : (ki + 1) * 128],
                            )
                            # diag blocks feed the bf16 denominator directly;
                            # under ones8 the first diag (covering [0:512])
                            # opens the accumulation group
                            nc.tensor.matmul(
                                sums[:, q0:512], ones_t[:, :], pp[:, q0:512],
                                start=(ki == (4 * qc if ones8 else 0)),
                                stop=(ki == nki_here - 1 and not (ones8 and qc)),
                            )
                        elif ones8:
                            if ki % 2 == 0:
                                pp8 = pp8p.tile([128, 2, 512], FP8, tag="pp8")
                                pp8s.append(pp8)
                            nc.vector.tensor_copy(pp8s[-1][:, ki % 2, :], pp[:, :])
                        else:
                            nc.tensor.matmul(
                                sums[:, :], ones_t[:, :], pp[:, :],
                                start=(ki == 0), stop=False,
                            )
                        nc.tensor.matmul(
                            avp[:, q0:512], v_sb[:, ki, hs], pp[:, q0:512],
                            start=(ki == 0), stop=(ki == nki_here - 1),
                        )
                    # deferred fp8 DoubleRow denominator over full-ki pairs:
                    # their vector copies completed long ago, so the in-order
                    # PE reaches these with no wait
                    for p, pp8 in enumerate(pp8s):
                        nc.tensor.matmul(
                            sums, ones2[:, :, :], pp8[:, :, :],
                            perf_mode=DR, start=False, stop=(p == len(pp8s) - 1),
                        )

                def attn_full(qc, h, sums, avp):
                    """non-causal fallback: every ki block, host-supplied mask."""
                    q_lo = qc * 512
                    hs = slice(h * 128, (h + 1) * 128)
                    for ki in range(NKI):
                        spbv = spp.tile([128, 512], F32, tag="sp")
                        pp = ppp.tile([128, 512], BF16, tag="ppf")
                        nc.tensor.matmul(
                            spbv[:, :],
                            k_rot[:, h, ki * 128 : (ki + 1) * 128],
                            q_rot[:, h, q_lo : q_lo + 512],
                            start=True, stop=True,
                        )
                        nc.scalar.activation(
                            pp[:, :], spbv[:, :], EXPF, scale=float(exp_scale)
                        )
                        mb = mblkp.tile([128, 512], BF16, tag="mblk")
                        nc.sync.dma_start(
                            out=mb[:, :], in_=maskf_v[:, ki, q_lo : q_lo + 512]
                        )
                        nc.vector.tensor_mul(pp[:, 0:512], pp[:, 0:512], mb[:, :])
                        nc.tensor.matmul(
                            sums[:, :], ones_t[:, :], pp[:, :],
                            start=(ki == 0), stop=(ki == NKI - 1),
                        )
                        nc.tensor.matmul(
                            avp[:, :], v_sb[:, ki, hs], pp[:, :],
                            start=(ki == 0), stop=(ki == NKI - 1),
                        )

                for qc in range(NQC):
                    q_lo = qc * 512
                    for h in range(HPC):
                        sav = accp.tile([128, 2, 512], F32, tag="acc")
                        sums = sav[:, 0, :]
                        avp = sav[:, 1, :]
                        if causal:
                            attn_causal(qc, h, sums, avp)
                        else:
                            attn_full(qc, h, sums, avp)
                        rc = rcp.tile([128, 512], F32, tag="rc")
                        nc.vector.reciprocal_approx_fast(rc[:, :], sums[:, :])
                        if o8h and h < 2:
                            ao_dst = aoT8[:, h, q_lo : q_lo + 512]
                        else:
                            ao_dst = aoT[:, h, q_lo : q_lo + 512]
                        nc.vector.tensor_mul(ao_dst, avp[:, :], rc[:, :])

                    # out-projection for this query chunk (4 seq blocks);
                    # both 1024-wide halves land in one tile -> one DMA per sb
                    for sb in range(4 * qc, 4 * qc + 4):
                        ss = slice(sb * 128, (sb + 1) * 128)
                        ob = osbp.tile([128, 2048], BF16, tag="osb")
                        for oc2 in range(2):
                            op2 = accp.tile([128, 2, 512], F32, tag="acc")
                            if o8:
                                for hp in range(2):
                                    for ocp in range(2):
                                        oc = 2 * oc2 + ocp
                                        os_ = slice(oc * 512, (oc + 1) * 512)
                                        nc.tensor.matmul(
                                            op2[:, ocp, :],
                                            aoT[:, 2 * hp : 2 * hp + 2, ss],
                                            wo[:, 2 * hp : 2 * hp + 2, os_],
                                            start=(hp == 0), stop=(hp == 1),
                                            perf_mode=DR,
                                        )
                            elif o8h:
                                # heads 0-1 as one fp8 DoubleRow pair,
                                # heads 2-3 bf16
                                for ocp in range(2):
                                    oc = 2 * oc2 + ocp
                                    os_ = slice(oc * 512, (oc + 1) * 512)
                                    nc.tensor.matmul(
                                        op2[:, ocp, :],
                                        aoT8[:, :, ss], wo8[:, :, os_],
                                        start=True, stop=False, perf_mode=DR,
                                    )
                                for hh in range(2, HPC):
                                    for ocp in range(2):
                                        oc = 2 * oc2 + ocp
                                        os_ = slice(oc * 512, (oc + 1) * 512)
                                        nc.tensor.matmul(
                                            op2[:, ocp, :],
                                            aoT[:, hh, ss], wo[:, hh, os_],
                                            start=False, stop=(hh == HPC - 1),
                                        )
                            else:
                                for hh in range(HPC):
                                    for ocp in range(2):
                                        oc = 2 * oc2 + ocp
                                        os_ = slice(oc * 512, (oc + 1) * 512)
                                        nc.tensor.matmul(
                                            op2[:, ocp, :],
                                            aoT[:, hh, ss], wo[:, hh, os_],
                                            start=(hh == 0), stop=(hh == HPC - 1),
                                        )
                            obh = ob[:, oc2 * 1024 : (oc2 + 1) * 1024]
                            if evict_flip[0] % 2 == 0:
                                nc.scalar.copy(obh, op2[:, :, :])
                            else:
                                nc.vector.tensor_copy(obh, op2[:, :, :])
                            evict_flip[0] += 1
                        nc.sync.dma_start(out=out_d[ss, :], in_=ob[:, :])

    nc.finalize()
    return nc


def _bit_quantize_ternary(w: np.ndarray):
    """Returns (ternary {-1,0,1} float32 matrix, scale) matching the reference."""
    scale = np.maximum(np.mean(np.abs(w.astype(np.float32))), np.float32(1e-5))
    t = np.clip(np.round(w.astype(np.float32) / scale), -1.0, 1.0).astype(np.float32)
    return t, float(scale)


def _host_tables():
    """cos/sin stacked [128, S]: rows 0:64 cos, rows 64:128 sin."""
    inv_freq = 1.0 / (ROPE_BASE ** (np.arange(0, D_HEAD, 2, dtype=np.float32) / D_HEAD))
    pos = np.arange(SEQ, dtype=np.float32)
    ang = pos[:, None] * inv_freq[None, :]  # [S, 64]
    cs = np.empty((128, SEQ), dtype=NPBF16)
    cs[0:64] = np.ascontiguousarray(np.cos(ang).T).astype(NPBF16)
    cs[64:128] = np.ascontiguousarray(np.sin(ang).T).astype(NPBF16)
    sc = np.empty((128, SEQ), dtype=NPBF16)
    sc[0:64] = cs[64:128]
    sc[64:128] = cs[0:64]
    return cs, sc


def kernel(x, w_qkv, w_out, mask):
    global LAST_RESULT
    x = np.asarray(x, dtype=np.float32)
    w_qkv = np.asarray(w_qkv, dtype=np.float32)
    w_out = np.asarray(w_out, dtype=np.float32)
    mask = np.asarray(mask)

    tq, sq = _bit_quantize_ternary(w_qkv)
    to, so = _bit_quantize_ternary(w_out)
    exp_scale = (sq * sq) / float(np.sqrt(D_HEAD))
    c2 = np.float32(sq * so)

    m2 = (mask.reshape(SEQ, SEQ) != 0).astype(np.float32)
    causal = bool(np.array_equal(m2, np.tril(np.ones((SEQ, SEQ), np.float32))))

    cfg = dict(CFG)
    o8 = cfg["o8"] and causal
    o8h = cfg["o8h"] and causal and not o8

    cs, sc = _host_tables()
    if causal:
        maskd = np.empty((128, SEQ), dtype=NPBF16)
        for ki in range(NKI):
            blk = m2[ki * 128 : (ki + 1) * 128, ki * 128 : (ki + 1) * 128]  # [q, k]
            maskd[:, ki * 128 : (ki + 1) * 128] = np.ascontiguousarray(blk.T).astype(
                NPBF16
            )
    else:
        maskf = np.ascontiguousarray(m2.T).astype(NPBF16)  # [kk, qq]

    key = (causal, float(exp_scale), tuple(sorted(cfg.items())))
    if key not in _PROG_CACHE:
        _PROG_CACHE[key] = _build_program(causal, float(exp_scale), cfg)
    nc = _PROG_CACHE[key]

    np_x = NPFP8 if cfg["a8"] else NPBF16
    np_wo = NPFP8 if o8 else NPBF16
    v2s = cfg["v2s"] and cfg["a8"]
    xTs, rTs = [], []
    for b in range(BATCH):
        xT = np.ascontiguousarray(x[b].T)
        x8 = xT.astype(np_x)
        xTs.append(x8)
        if v2s:
            rTs.append((xT - x8.astype(np.float32)).astype(NPFP8))
    in_maps = []
    for c in range(N_CORES):
        b, g = divmod(c, 4)
        rows = slice(R_LOCAL * g, R_LOCAL * (g + 1))
        woT_np = np.ascontiguousarray(to[:, rows].T)
        im = {
            "xT": xTs[b],
            "wqT": np.ascontiguousarray(tq[0 * D_MODEL :][rows].T).astype(NPFP8),
            "wkT": np.ascontiguousarray(tq[1 * D_MODEL :][rows].T).astype(NPFP8),
            "wvT": np.ascontiguousarray(tq[2 * D_MODEL :][rows].T).astype(NPFP8),
            "woT": woT_np.astype(np_wo),
            "cossinT": cs,
            "sincosT": sc,
        }
        if o8h:
            im["woT8"] = np.ascontiguousarray(woT_np[0 : 2 * D_HEAD]).astype(NPFP8)
        if v2s:
            im["rT"] = rTs[b]
        if causal:
            im["maskd"] = maskd
        else:
            im["maskf"] = maskf
        in_maps.append(im)

    do_trace = bool(PROFILE) and _enable_profiling()
    res = run_bass_kernel_spmd(nc, in_maps, list(range(N_CORES)), trace=do_trace)
    LAST_RESULT = res

    parts = [np.asarray(res.results[c]["out"]).astype(np.float32) for c in range(N_CORES)]
    out = np.stack(
        [
            parts[0] + parts[1] + parts[2] + parts[3],
            parts[4] + parts[5] + parts[6] + parts[7],
        ]
    )
    return (out * c2).astype(np.float32)


# revision 54
# speedup vs baseline: 1.0396x; 1.0396x over previous
"""Trainium2 Bass kernel for BitNet multi-head attention (nn_MultiHeadAttention_62294205661880).

Sharding: 8 cores = 2 batches x 4 head-groups (4 heads each).  Each core
computes qkv projection, RoPE, causal attention and a column-parallel slice
of the output projection for its (batch, head-group); the host sums the 4
partial out-projections per batch.

BitNet quantization is folded on the host: weights are uploaded as exact
ternary {-1,0,+1} fp8 matrices; scale_qkv^2/sqrt(dh) is folded into the
softmax exp() scale and scale_qkv*scale_out into a final host-side scalar.

FP8 acceleration (per-stage flags in CFG).  fp8 DoubleRow matmuls contract
two 128-row k-tiles per pass (2x flops/cycle vs bf16); measured error budget
(rel err vs 2e-2 gate): x->fp8 noise on the q/k path costs ~0.007 rel, on
the v path ~0.017 (blocked), so:
  a8   - q/k projections from 1-stream fp8 x (fast, small scores-path noise)
  v2s  - v projection from x8 + fp8-residual 2-stream (bf16-accurate, bf16
         speed, but keeps the whole phase on the fp8 DoubleRow pipeline)
  ones8- softmax denominator: gpsimd copies pp to fp8, the all-ones
         stationary matmul runs DoubleRow over ki-block pairs (denominator
         averages ~128+ keys so pp quantization noise is negligible there)
  o8   - (optional nibble) out-projection aoT in fp8 -- costs ~0.017 rel,
         disabled by default

Device layout trick: everything is computed transposed.  Q_T/K_T come out of
the projection as [dh, S]; scores are computed as s_T[k, q]; the softmax
denominator sums over the partition (key) dim via an all-ones stationary
matmul; AV produces out_T[dh, q] which feeds the output projection directly.
Softmax skips the max-subtraction: scores are bounded (~+-2) because the
BitNet weight scale is tiny, so exp() is safe -- this also makes it safe to
run exp over never-written PSUM regions (any stale f32 times the ~2e-5 exp
scale stays finite), which lets one ACT instruction cover a pair of
diagonal score blocks whose valid column ranges differ.
"""

import sys
import types

import numpy as np
import ml_dtypes

import concourse.bass as bass
import concourse.mybir as mybir
import concourse.tile as tile
from concourse import bacc
from concourse.bass_utils import run_bass_kernel_spmd

D_MODEL = 2048
N_HEADS = 16
D_HEAD = 128
SEQ = 2048
BATCH = 2
ROPE_BASE = 10000.0

N_CORES = 8
HPC = 4  # heads per core
R_LOCAL = HPC * D_HEAD  # 512 local q (or k, or v) rows per core
MO = D_MODEL // 128  # 16 contraction blocks
NKI = SEQ // 128  # 16 key blocks
NQC = SEQ // 512  # 4 query chunks of 512
NSB = SEQ // 128  # 16 seq blocks (v / proj)

BF16 = mybir.dt.bfloat16
F32 = mybir.dt.float32
NPBF16 = ml_dtypes.bfloat16
NPFP8 = ml_dtypes.float8_e4m3
FP8 = mybir.dt.float8e4
DR = mybir.MatmulPerfMode.DoubleRow

# o8h: out-projection aoT in fp8 for heads 0-1 only (one DoubleRow pair),
#      heads 2-3 bf16 — half the aoT quantization noise of full o8
CFG = dict(a8=True, v2s=True, ones8=False, o8=False, o8h=True, rope4=False)

LAST_RESULT = None  # BassKernelResults of the most recent run (for test.py)
_PROG_CACHE = {}
PROFILE = False  # test.py sets True to capture an NTFF profile / HW exec time


def _enable_profiling() -> bool:
    """Install the axon NTFF profile hook glue if the image lacks
    ``antenv.axon_hooks`` (boot degrades silently without it), and skip
    the artifact upload (no bucket access in this container)."""
    try:
        from antenv.axon_hooks import get_axon_ntff_profile_hook  # noqa: F401

        ok = get_axon_ntff_profile_hook() is not None
    except ImportError:
        ok = False
        import antenv

        mod = types.ModuleType("antenv.axon_hooks")
        mod._hook = None
        mod.set_axon_ntff_profile_hook = lambda h: setattr(mod, "_hook", h)
        mod.get_axon_ntff_profile_hook = lambda: mod._hook
        sys.modules["antenv.axon_hooks"] = mod
        antenv.axon_hooks = mod
        try:
            from trn_agent_boot.trn_boot import _ntff_profile_via_ctypes

            hook = _ntff_profile_via_ctypes("/opt/axon/libaxon_pjrt.so")
            if hook is not None:
                mod._hook = hook
                ok = True
        except Exception as e:  # profiling is best-effort
            print(f"ntff profile hook install failed: {e}", file=sys.stderr)
    if ok:
        import concourse.bass_utils as _bu

        _bu.upload_artifacts = lambda tmpdir: tmpdir
    return ok


def _build_program(causal: bool, exp_scale: float, cfg: dict) -> bass.Bass:
    a8 = cfg["a8"]
    v2s = cfg["v2s"] and a8
    ones8 = cfg["ones8"] and causal
    o8 = cfg["o8"] and causal
    o8h = cfg["o8h"] and causal and not o8

    nc = bacc.Bacc(None)
    S = SEQ

    x_dt = FP8 if a8 else BF16
    wo_dt = FP8 if o8 else BF16
    ao_dt = FP8 if o8 else BF16

    xT_d = nc.dram_tensor("xT", [D_MODEL, S], x_dt, kind="ExternalInput")
    if v2s:
        rT_d = nc.dram_tensor("rT", [D_MODEL, S], FP8, kind="ExternalInput")
    wqT_d = nc.dram_tensor("wqT", [D_MODEL, R_LOCAL], FP8, kind="ExternalInput")
    wkT_d = nc.dram_tensor("wkT", [D_MODEL, R_LOCAL], FP8, kind="ExternalInput")
    wvT_d = nc.dram_tensor("wvT", [D_MODEL, R_LOCAL], FP8, kind="ExternalInput")
    woT_d = nc.dram_tensor("woT", [R_LOCAL, D_MODEL], wo_dt, kind="ExternalInput")
    if o8h:
        woT8_d = nc.dram_tensor(
            "woT8", [2 * D_HEAD, D_MODEL], FP8, kind="ExternalInput"
        )
    # cos rows 0:64, sin rows 64:128
    cs_d = nc.dram_tensor("cossinT", [128, S], BF16, kind="ExternalInput")
    # swapped: sin rows 0:64, cos rows 64:128 (keeps TensorTensor base partitions equal)
    sc_d = nc.dram_tensor("sincosT", [128, S], BF16, kind="ExternalInput")
    if causal:
        # 16 transposed diagonal 128x128 mask blocks, side by side
        maskd_d = nc.dram_tensor("maskd", [128, S], BF16, kind="ExternalInput")
    else:
        maskf_d = nc.dram_tensor("maskf", [S, S], BF16, kind="ExternalInput")
    out_d = nc.dram_tensor("out", [S, D_MODEL], BF16, kind="ExternalOutput")

    xT_v = xT_d[:].rearrange("(mo p) s -> p mo s", p=128)
    if v2s:
        rT_v = rT_d[:].rearrange("(mo p) s -> p mo s", p=128)
    wqT_v = wqT_d[:].rearrange("(mo p) r -> p mo r", p=128)
    wkT_v = wkT_d[:].rearrange("(mo p) r -> p mo r", p=128)
    wvT_v = wvT_d[:].rearrange("(mo p) r -> p mo r", p=128)
    woT_v = woT_d[:].rearrange("(h p) o -> p h o", p=128)
    if o8h:
        woT8_v = woT8_d[:].rearrange("(h p) o -> p h o", p=128)
    if not causal:
        maskf_v = maskf_d[:].rearrange("(ko p) q -> p ko q", p=128)

    with tile.TileContext(nc) as tc:
        with tc.tile_pool(name="pers", bufs=1) as pers:
            # ---- persistent SBUF tensors (live across both phases) ----
            q_rot = pers.tile([128, HPC, S], BF16, tag="qrot")
            k_rot = pers.tile([128, HPC, S], BF16, tag="krot")
            v_sb = pers.tile([128, NKI, R_LOCAL], BF16, tag="vsb")
            aoT = pers.tile([128, HPC, S], ao_dt, tag="aoT")
            wo = pers.tile([128, HPC, D_MODEL], wo_dt, tag="wo")
            if o8h:
                aoT8 = pers.tile([128, 2, S], FP8, tag="aoT8")
                wo8 = pers.tile([128, 2, D_MODEL], FP8, tag="wo8")
            ones_t = pers.tile([128, 128], BF16, tag="ones")
            if ones8:
                ones2 = pers.tile([128, 2, 128], FP8, tag="ones2")
                nc.vector.memset(ones2[:], 1.0)
            warm = pers.tile([128, 2], BF16, tag="warm")
            if causal:
                maskd = pers.tile([128, S], BF16, tag="maskd")
            nc.vector.memset(ones_t[:, :], 1.0)
            # load the exp table set first so no ACT table switch happens
            # mid-kernel (Copy lives in every set).
            nc.vector.memset(warm[:, 0:1], 1.0)
            nc.scalar.activation(
                warm[:, 1:2], warm[:, 0:1], mybir.ActivationFunctionType.Exp
            )

            # ================= phase A: QKV projection + RoPE =================
            with (
                tc.tile_pool(name="xtp", bufs=1) as xtp,
                tc.tile_pool(name="wp", bufs=1) as wp,
                tc.tile_pool(name="raw", bufs=2) as rawp,
                tc.tile_pool(name="w8", bufs=6) as w8p,
                tc.tile_pool(name="tmp", bufs=2) as tmpp,
                tc.tile_pool(name="psA", bufs=4, space="PSUM") as psA,
            ):
                w_dt = FP8 if a8 else BF16
                xt = xtp.tile([128, MO, S], x_dt, tag="xt")
                if v2s:
                    rt = xtp.tile([128, MO, S], FP8, tag="rt")
                wq = wp.tile([128, MO, R_LOCAL], w_dt, tag="wq")
                wk = wp.tile([128, MO, R_LOCAL], w_dt, tag="wk")
                wv = wp.tile([128, MO, R_LOCAL], w_dt, tag="wv")
                cs_t = wp.tile([128, S], BF16, tag="cs")
                sc_t = wp.tile([128, S], BF16, tag="sc")

                def load_w(dst, view, mo4):
                    """load weight m-blocks mo4..mo4+3 (fp8 direct or cast)."""
                    if a8:  # straight fp8 DMA, no cast
                        nc.sync.dma_start(
                            out=dst[:, mo4 : mo4 + 4, :], in_=view[:, mo4 : mo4 + 4, :]
                        )
                    else:
                        for mo in range(mo4, mo4 + 4):
                            st = w8p.tile([128, R_LOCAL], FP8, tag="w8")
                            nc.sync.dma_start(out=st[:, :], in_=view[:, mo, :])
                            nc.vector.tensor_copy(dst[:, mo, :], st[:, :])

                # critical path first: wq chunk 0 + first x pairs feed the
                # h=0 q projection; w chunks and rope tables land ahead of
                # the bulk x blocks they gate.
                # each dma_start costs ~0.64us of serialized issue time on the
                # sync engine, so phase A uses few, chunky transfers; the DMA
                # queue is FIFO on transfers, so weight chunks interleave with
                # the x bulk (front-loading x delays the wk chunks the h=0
                # k-projection needs and measures slower).
                load_w(wq, wqT_v, 0)
                nc.sync.dma_start(out=xt[:, 0:2, 0:1024], in_=xT_v[:, 0:2, 0:1024])
                load_w(wk, wkT_v, 0)
                nc.sync.dma_start(
                    out=xt[:, 0:2, 1024:2048], in_=xT_v[:, 0:2, 1024:2048]
                )
                nc.sync.dma_start(out=xt[:, 2:4, :], in_=xT_v[:, 2:4, :])
                for c4m in range(1, 4):
                    load_w(wq, wqT_v, 4 * c4m)
                    load_w(wk, wkT_v, 4 * c4m)
                    nc.sync.dma_start(
                        out=xt[:, 4 * c4m : 4 * c4m + 4, :],
                        in_=xT_v[:, 4 * c4m : 4 * c4m + 4, :],
                    )
                # rope tables ride behind the x bulk: rope runs on the
                # (slack) vector engine, while a table transfer queued
                # mid-x stalls the tensor engine's m-pair consumption
                nc.sync.dma_start(out=cs_t[:, :], in_=cs_d[:, :])
                nc.sync.dma_start(out=sc_t[:, :], in_=sc_d[:, :])
                for c4m in range(4):
                    load_w(wv, wvT_v, 4 * c4m)
                    if v2s:
                        nc.sync.dma_start(
                            out=rt[:, 4 * c4m : 4 * c4m + 4, :],
                            in_=rT_v[:, 4 * c4m : 4 * c4m + 4, :],
                        )
                if causal:  # first needed in phase B
                    nc.sync.dma_start(out=maskd[:, :], in_=maskd_d[:, :])
                # prefetch the out-projection weights behind everything else
                # so phase B starts without a DMA wait
                if o8h:
                    nc.sync.dma_start(out=wo8[:, :, :], in_=woT8_v[:, :, :])
                    nc.sync.dma_start(out=wo[:, 2:4, :], in_=woT_v[:, 2:4, :])
                else:
                    for oc in range(D_MODEL // 512):
                        nc.sync.dma_start(
                            out=wo[:, :, oc * 512 : (oc + 1) * 512],
                            in_=woT_v[:, :, oc * 512 : (oc + 1) * 512],
                        )

                def rope(h, raw, dst):
                    """NeoX rotary: low = t1*c - t2*s ; hi = t1*s + t2*c."""
                    d_lo = dst[0:64, h, :]
                    d_hi = dst[64:128, h, :]
                    if cfg["rope4"]:
                        # 4 full-width ops, cross-base-partition sub/add
                        p1 = tmpp.tile([128, S], BF16, tag="tmp")
                        p2 = tmpp.tile([128, S], BF16, tag="tmp")
                        nc.vector.tensor_mul(p1[:, :], raw[:, :], cs_t[:, :])
                        nc.vector.tensor_mul(p2[:, :], raw[:, :], sc_t[:, :])
                        nc.vector.tensor_sub(d_lo, p1[0:64, :], p1[64:128, :])
                        nc.vector.tensor_add(d_hi, p2[0:64, :], p2[64:128, :])
                    else:
                        ta = tmpp.tile([64, S], BF16, tag="tmp")
                        tb = tmpp.tile([64, S], BF16, tag="tmp")
                        nc.vector.tensor_mul(ta[:, :], raw[0:64, :], cs_t[0:64, :])
                        nc.vector.tensor_mul(tb[:, :], raw[64:128, :], cs_t[64:128, :])
                        nc.vector.tensor_sub(d_lo, ta[:, :], tb[:, :])
                        tc2 = tmpp.tile([64, S], BF16, tag="tmp")
                        td = tmpp.tile([64, S], BF16, tag="tmp")
                        nc.vector.tensor_mul(tc2[:, :], raw[0:64, :], sc_t[0:64, :])
                        nc.vector.tensor_mul(td[:, :], raw[64:128, :], sc_t[64:128, :])
                        nc.vector.tensor_add(d_hi, tc2[:, :], td[:, :])

                def project_qk(h):
                    # two half-width psum tiles per projection: finer WAR
                    # rotation lets the next head's matmuls start while this
                    # head's second half is still being evicted
                    psq0 = psA.tile([128, 1024], F32, tag="psA")
                    psq1 = psA.tile([128, 1024], F32, tag="psA")
                    psk0 = psA.tile([128, 1024], F32, tag="psA")
                    psk1 = psA.tile([128, 1024], F32, tag="psA")
                    hs = slice(h * 128, (h + 1) * 128)

                    def mm_into(halves, w_sb, m_sl, first, last, pm):
                        for c4 in range(4):
                            cs_ = slice(c4 * 512, (c4 + 1) * 512)
                            tgt = halves[c4 // 2][:, (c4 % 2) * 512 : (c4 % 2) * 512 + 512]
                            nc.tensor.matmul(
                                tgt, w_sb[:, m_sl, hs], xt[:, m_sl, cs_],
                                start=first, stop=last, perf_mode=pm,
                            )

                    if a8:
                        for m2 in range(MO // 2):
                            ms = slice(2 * m2, 2 * m2 + 2)
                            mm_into((psq0, psq1), wq, ms, m2 == 0, m2 == MO // 2 - 1, DR)
                            mm_into((psk0, psk1), wk, ms, m2 == 0, m2 == MO // 2 - 1, DR)
                    else:
                        for m in range(MO):
                            mm_into((psq0, psq1), wq, m, m == 0, m == MO - 1, None)
                            mm_into((psk0, psk1), wk, m, m == 0, m == MO - 1, None)
                    q_raw = rawp.tile([128, S], BF16, tag="raw")
                    nc.scalar.copy(q_raw[:, 0:1024], psq0[:, :])
                    nc.scalar.copy(q_raw[:, 1024:2048], psq1[:, :])
                    rope(h, q_raw, q_rot)
                    k_raw = rawp.tile([128, S], BF16, tag="raw")
                    nc.scalar.copy(k_raw[:, 0:1024], psk0[:, :])
                    nc.scalar.copy(k_raw[:, 1024:2048], psk1[:, :])
                    rope(h, k_raw, k_rot)

                for h in range(HPC):
                    project_qk(h)

                # ---- V projection (natural layout [s, r]) ----
                for sb2 in range(NSB // 2):
                    ps = psA.tile([128, 1024], F32, tag="psA")
                    for part in range(2):
                        sb = sb2 * 2 + part
                        ss = slice(sb * 128, (sb + 1) * 128)
                        ps_ = ps[:, part * 512 : part * 512 + 512]
                        if a8:
                            for m2 in range(MO // 2):
                                ms = slice(2 * m2, 2 * m2 + 2)
                                nc.tensor.matmul(
                                    ps_, xt[:, ms, ss], wv[:, ms, :],
                                    start=(m2 == 0),
                                    stop=(not v2s and m2 == MO // 2 - 1),
                                    perf_mode=DR,
                                )
                            if v2s:  # residual stream restores bf16 accuracy
                                for m2 in range(MO // 2):
                                    ms = slice(2 * m2, 2 * m2 + 2)
                                    nc.tensor.matmul(
                                        ps_, rt[:, ms, ss], wv[:, ms, :],
                                        start=False, stop=(m2 == MO // 2 - 1),
                                        perf_mode=DR,
                                    )
                        else:
                            for m in range(MO):
                                nc.tensor.matmul(
                                    ps_, xt[:, m, ss], wv[:, m, :],
                                    start=(m == 0), stop=(m == MO - 1),
                                )
                    dst = v_sb[:, sb2 * 2 : sb2 * 2 + 2, :]
                    nc.scalar.copy(dst, ps[:, :])

            # ================= phase B: attention + out-projection =============
            with (
                tc.tile_pool(name="pp", bufs=8) as ppp,
                tc.tile_pool(name="pp8", bufs=8) as pp8p,
                tc.tile_pool(name="rcp", bufs=3) as rcp,
                tc.tile_pool(name="osb", bufs=4) as osbp,
                tc.tile_pool(name="mblk", bufs=4) as mblkp,
                tc.tile_pool(name="sp", bufs=4, space="PSUM") as spp,
                tc.tile_pool(name="acc", bufs=2, space="PSUM") as accp,
            ):
                evict_flip = [0]
                EXPF = mybir.ActivationFunctionType.Exp

                def attn_causal(qc, h, sums, avp):
                    """per-ki pipeline; full-block denominators collected as
                    fp8 pairs and summed by deferred DoubleRow matmuls."""
                    q_lo = qc * 512
                    hs = slice(h * 128, (h + 1) * 128)
                    nki_here = 4 * qc + 4
                    pp8s = []
                    for ki in range(nki_here):
                        diag = ki >= 4 * qc
                        q0 = 128 * (ki - 4 * qc) if diag else 0
                        spbv = spp.tile([128, 512], F32, tag="sp")
                        pp = ppp.tile([128, 512], BF16, tag="pp")
                        nc.tensor.matmul(
                            spbv[:, q0:512],
                            k_rot[:, h, ki * 128 : (ki + 1) * 128],
                            q_rot[:, h, q_lo + q0 : q_lo + 512],
                            start=True, stop=True,
                        )
                        nc.scalar.activation(
                            pp[:, q0:512], spbv[:, q0:512], EXPF,
                            scale=float(exp_scale),
                        )
                        if diag:
                            nc.vector.tensor_mul(
                                pp[:, q0 : q0 + 128], pp[:, q0 : q0 + 128],
                                maskd[:, ki * 128 : (ki + 1) * 128],
                            )
                            # diag blocks feed the bf16 denominator directly;
                            # under ones8 the first diag (covering [0:512])
                            # opens the accumulation group
                            nc.tensor.matmul(
                                sums[:, q0:512], ones_t[:, :], pp[:, q0:512],
                                start=(ki == (4 * qc if ones8 else 0)),
                                stop=(ki == nki_here - 1 and not (ones8 and qc)),
                            )
                        elif ones8:
                            if ki % 2 == 0:
                                pp8 = pp8p.tile([128, 2, 512], FP8, tag="pp8")
                                pp8s.append(pp8)
                            nc.vector.tensor_copy(pp8s[-1][:, ki % 2, :], pp[:, :])
                        else:
                            nc.tensor.matmul(
                                sums[:, :], ones_t[:, :], pp[:, :],
                                start=(ki == 0), stop=False,
                            )
                        nc.tensor.matmul(
                            avp[:, q0:512], v_sb[:, ki, hs], pp[:, q0:512],
                            start=(ki == 0), stop=(ki == nki_here - 1),
                        )
                    # deferred fp8 DoubleRow denominator over full-ki pairs:
                    # their vector copies completed long ago, so the in-order
                    # PE reaches these with no wait
                    for p, pp8 in enumerate(pp8s):
                        nc.tensor.matmul(
                            sums, ones2[:, :, :], pp8[:, :, :],
                            perf_mode=DR, start=False, stop=(p == len(pp8s) - 1),
                        )

                def attn_full(qc, h, sums, avp):
                    """non-causal fallback: every ki block, host-supplied mask."""
                    q_lo = qc * 512
                    hs = slice(h * 128, (h + 1) * 128)
                    for ki in range(NKI):
                        spbv = spp.tile([128, 512], F32, tag="sp")
                        pp = ppp.tile([128, 512], BF16, tag="ppf")
                        nc.tensor.matmul(
                            spbv[:, :],
                            k_rot[:, h, ki * 128 : (ki + 1) * 128],
                            q_rot[:, h, q_lo : q_lo + 512],
                            start=True, stop=True,
                        )
                        nc.scalar.activation(
                            pp[:, :], spbv[:, :], EXPF, scale=float(exp_scale)
                        )
                        mb = mblkp.tile([128, 512], BF16, tag="mblk")
                        nc.sync.dma_start(
                            out=mb[:, :], in_=maskf_v[:, ki, q_lo : q_lo + 512]
                        )
                        nc.vector.tensor_mul(pp[:, 0:512], pp[:, 0:512], mb[:, :])
                        nc.tensor.matmul(
                            sums[:, :], ones_t[:, :], pp[:, :],
                            start=(ki == 0), stop=(ki == NKI - 1),
                        )
                        nc.tensor.matmul(
                            avp[:, :], v_sb[:, ki, hs], pp[:, :],
                            start=(ki == 0), stop=(ki == NKI - 1),
                        )

                for qc in range(NQC):
                    q_lo = qc * 512
                    for h in range(HPC):
                        sav = accp.tile([128, 2, 512], F32, tag="acc")
                        sums = sav[:, 0, :]
                        avp = sav[:, 1, :]
                        if causal:
                            attn_causal(qc, h, sums, avp)
                        else:
                            attn_full(qc, h, sums, avp)
                        rc = rcp.tile([128, 512], F32, tag="rc")
                        nc.vector.reciprocal_approx_fast(rc[:, :], sums[:, :])
                        if o8h and h < 2:
                            ao_dst = aoT8[:, h, q_lo : q_lo + 512]
                        else:
                            ao_dst = aoT[:, h, q_lo : q_lo + 512]
                        nc.vector.tensor_mul(ao_dst, avp[:, :], rc[:, :])

                    # out-projection for this query chunk (4 seq blocks);
                    # both 1024-wide halves land in one tile -> one DMA per sb
                    for sb in range(4 * qc, 4 * qc + 4):
                        ss = slice(sb * 128, (sb + 1) * 128)
                        ob = osbp.tile([128, 2048], BF16, tag="osb")
                        for oc2 in range(2):
                            op2 = accp.tile([128, 2, 512], F32, tag="acc")
                            if o8:
                                for hp in range(2):
                                    for ocp in range(2):
                                        oc = 2 * oc2 + ocp
                                        os_ = slice(oc * 512, (oc + 1) * 512)
                                        nc.tensor.matmul(
                                            op2[:, ocp, :],
                                            aoT[:, 2 * hp : 2 * hp + 2, ss],
                                            wo[:, 2 * hp : 2 * hp + 2, os_],
                                            start=(hp == 0), stop=(hp == 1),
                                            perf_mode=DR,
                                        )
                            elif o8h:
                                # heads 0-1 as one fp8 DoubleRow pair,
                                # heads 2-3 bf16
                                for ocp in range(2):
                                    oc = 2 * oc2 + ocp
                                    os_ = slice(oc * 512, (oc + 1) * 512)
                                    nc.tensor.matmul(
                                        op2[:, ocp, :],
                                        aoT8[:, :, ss], wo8[:, :, os_],
                                        start=True, stop=False, perf_mode=DR,
                                    )
                                for hh in range(2, HPC):
                                    for ocp in range(2):
                                        oc = 2 * oc2 + ocp
                                        os_ = slice(oc * 512, (oc + 1) * 512)
                                        nc.tensor.matmul(
                                            op2[:, ocp, :],
                                            aoT[:, hh, ss], wo[:, hh, os_],
                                            start=False, stop=(hh == HPC - 1),
                                        )
                            else:
                                for hh in range(HPC):
                                    for ocp in range(2):
                                        oc = 2 * oc2 + ocp
                                        os_ = slice(oc * 512, (oc + 1) * 512)
                                        nc.tensor.matmul(
                                            op2[:, ocp, :],
                                            aoT[:, hh, ss], wo[:, hh, os_],
                                            start=(hh == 0), stop=(hh == HPC - 1),
                                        )
                            obh = ob[:, oc2 * 1024 : (oc2 + 1) * 1024]
                            if evict_flip[0] % 2 == 0:
                                nc.scalar.copy(obh, op2[:, :, :])
                            else:
                                nc.vector.tensor_copy(obh, op2[:, :, :])
                            evict_flip[0] += 1
                        nc.sync.dma_start(out=out_d[ss, :], in_=ob[:, :])

    nc.finalize()
    return nc


def _bit_quantize_ternary(w: np.ndarray):
    """Returns (ternary {-1,0,1} float32 matrix, scale) matching the reference."""
    scale = np.maximum(np.mean(np.abs(w.astype(np.float32))), np.float32(1e-5))
    t = np.clip(np.round(w.astype(np.float32) / scale), -1.0, 1.0).astype(np.float32)
    return t, float(scale)


def _host_tables():
    """cos/sin stacked [128, S]: rows 0:64 cos, rows 64:128 sin."""
    inv_freq = 1.0 / (ROPE_BASE ** (np.arange(0, D_HEAD, 2, dtype=np.float32) / D_HEAD))
    pos = np.arange(SEQ, dtype=np.float32)
    ang = pos[:, None] * inv_freq[None, :]  # [S, 64]
    cs = np.empty((128, SEQ), dtype=NPBF16)
    cs[0:64] = np.ascontiguousarray(np.cos(ang).T).astype(NPBF16)
    cs[64:128] = np.ascontiguousarray(np.sin(ang).T).astype(NPBF16)
    sc = np.empty((128, SEQ), dtype=NPBF16)
    sc[0:64] = cs[64:128]
    sc[64:128] = cs[0:64]
    return cs, sc


def kernel(x, w_qkv, w_out, mask):
    global LAST_RESULT
    x = np.asarray(x, dtype=np.float32)
    w_qkv = np.asarray(w_qkv, dtype=np.float32)
    w_out = np.asarray(w_out, dtype=np.float32)
    mask = np.asarray(mask)

    tq, sq = _bit_quantize_ternary(w_qkv)
    to, so = _bit_quantize_ternary(w_out)
    exp_scale = (sq * sq) / float(np.sqrt(D_HEAD))
    c2 = np.float32(sq * so)

    m2 = (mask.reshape(SEQ, SEQ) != 0).astype(np.float32)
    causal = bool(np.array_equal(m2, np.tril(np.ones((SEQ, SEQ), np.float32))))

    cfg = dict(CFG)
    o8 = cfg["o8"] and causal
    o8h = cfg["o8h"] and causal and not o8

    cs, sc = _host_tables()
    if causal:
        maskd = np.empty((128, SEQ), dtype=NPBF16)
        for ki in range(NKI):
            blk = m2[ki * 128 : (ki + 1) * 128, ki * 128 : (ki + 1) * 128]  # [q, k]
            maskd[:, ki * 128 : (ki + 1) * 128] = np.ascontiguousarray(blk.T).astype(
                NPBF16
            )
    else:
        maskf = np.ascontiguousarray(m2.T).astype(NPBF16)  # [kk, qq]

    key = (causal, float(exp_scale), tuple(sorted(cfg.items())))
    if key not in _PROG_CACHE:
        _PROG_CACHE[key] = _build_program(causal, float(exp_scale), cfg)
    nc = _PROG_CACHE[key]

    np_x = NPFP8 if cfg["a8"] else NPBF16
    np_wo = NPFP8 if o8 else NPBF16
    v2s = cfg["v2s"] and cfg["a8"]
    xTs, rTs = [], []
    for b in range(BATCH):
        xT = np.ascontiguousarray(x[b].T)
        x8 = xT.astype(np_x)
        xTs.append(x8)
        if v2s:
            rTs.append((xT - x8.astype(np.float32)).astype(NPFP8))
    in_maps = []
    for c in range(N_CORES):
        b, g = divmod(c, 4)
        rows = slice(R_LOCAL * g, R_LOCAL * (g + 1))
        woT_np = np.ascontiguousarray(to[:, rows].T)
        im = {
            "xT": xTs[b],
            "wqT": np.ascontiguousarray(tq[0 * D_MODEL :][rows].T).astype(NPFP8),
            "wkT": np.ascontiguousarray(tq[1 * D_MODEL :][rows].T).astype(NPFP8),
            "wvT": np.ascontiguousarray(tq[2 * D_MODEL :][rows].T).astype(NPFP8),
            "woT": woT_np.astype(np_wo),
            "cossinT": cs,
            "sincosT": sc,
        }
        if o8h:
            im["woT8"] = np.ascontiguousarray(woT_np[0 : 2 * D_HEAD]).astype(NPFP8)
        if v2s:
            im["rT"] = rTs[b]
        if causal:
            im["maskd"] = maskd
        else:
            im["maskf"] = maskf
        in_maps.append(im)

    do_trace = bool(PROFILE) and _enable_profiling()
    res = run_bass_kernel_spmd(nc, in_maps, list(range(N_CORES)), trace=do_trace)
    LAST_RESULT = res

    parts = [np.asarray(res.results[c]["out"]).astype(np.float32) for c in range(N_CORES)]
    out = np.stack(
        [
            parts[0] + parts[1] + parts[2] + parts[3],
            parts[4] + parts[5] + parts[6] + parts[7],
        ]
    )
    return (out * c2).astype(np.float32)


# revision 56
# speedup vs baseline: 1.0401x; 1.0004x over previous
"""Trainium2 Bass kernel for BitNet multi-head attention (nn_MultiHeadAttention_62294205661880).

Sharding: 8 cores = 2 batches x 4 head-groups (4 heads each).  Each core
computes qkv projection, RoPE, causal attention and a column-parallel slice
of the output projection for its (batch, head-group); the host sums the 4
partial out-projections per batch.

BitNet quantization is folded on the host: weights are uploaded as exact
ternary {-1,0,+1} fp8 matrices; scale_qkv^2/sqrt(dh) is folded into the
softmax exp() scale and scale_qkv*scale_out into a final host-side scalar.

FP8 acceleration (per-stage flags in CFG).  fp8 DoubleRow matmuls contract
two 128-row k-tiles per pass (2x flops/cycle vs bf16); measured error budget
(rel err vs 2e-2 gate): x->fp8 noise on the q/k path costs ~0.007 rel, on
the v path ~0.017 (blocked), so:
  a8   - q/k projections from 1-stream fp8 x (fast, small scores-path noise)
  v2s  - v projection from x8 + fp8-residual 2-stream (bf16-accurate, bf16
         speed, but keeps the whole phase on the fp8 DoubleRow pipeline)
  ones8- softmax denominator: gpsimd copies pp to fp8, the all-ones
         stationary matmul runs DoubleRow over ki-block pairs (denominator
         averages ~128+ keys so pp quantization noise is negligible there)
  o8   - (optional nibble) out-projection aoT in fp8 -- costs ~0.017 rel,
         disabled by default

Device layout trick: everything is computed transposed.  Q_T/K_T come out of
the projection as [dh, S]; scores are computed as s_T[k, q]; the softmax
denominator sums over the partition (key) dim via an all-ones stationary
matmul; AV produces out_T[dh, q] which feeds the output projection directly.
Softmax skips the max-subtraction: scores are bounded (~+-2) because the
BitNet weight scale is tiny, so exp() is safe -- this also makes it safe to
run exp over never-written PSUM regions (any stale f32 times the ~2e-5 exp
scale stays finite), which lets one ACT instruction cover a pair of
diagonal score blocks whose valid column ranges differ.
"""

import sys
import types

import numpy as np
import ml_dtypes

import concourse.bass as bass
import concourse.mybir as mybir
import concourse.tile as tile
from concourse import bacc
from concourse.bass_utils import run_bass_kernel_spmd

D_MODEL = 2048
N_HEADS = 16
D_HEAD = 128
SEQ = 2048
BATCH = 2
ROPE_BASE = 10000.0

N_CORES = 8
HPC = 4  # heads per core
R_LOCAL = HPC * D_HEAD  # 512 local q (or k, or v) rows per core
MO = D_MODEL // 128  # 16 contraction blocks
NKI = SEQ // 128  # 16 key blocks
NQC = SEQ // 512  # 4 query chunks of 512
NSB = SEQ // 128  # 16 seq blocks (v / proj)

BF16 = mybir.dt.bfloat16
F32 = mybir.dt.float32
NPBF16 = ml_dtypes.bfloat16
NPFP8 = ml_dtypes.float8_e4m3
FP8 = mybir.dt.float8e4
DR = mybir.MatmulPerfMode.DoubleRow

# o8h: out-projection aoT in fp8 for heads 0-1 only (one DoubleRow pair),
#      heads 2-3 bf16 — half the aoT quantization noise of full o8
CFG = dict(a8=True, v2s=True, ones8=False, o8=False, o8h=True, rope4=False)

LAST_RESULT = None  # BassKernelResults of the most recent run (for test.py)
_PROG_CACHE = {}
PROFILE = False  # test.py sets True to capture an NTFF profile / HW exec time


def _enable_profiling() -> bool:
    """Install the axon NTFF profile hook glue if the image lacks
    ``antenv.axon_hooks`` (boot degrades silently without it), and skip
    the artifact upload (no bucket access in this container)."""
    try:
        from antenv.axon_hooks import get_axon_ntff_profile_hook  # noqa: F401

        ok = get_axon_ntff_profile_hook() is not None
    except ImportError:
        ok = False
        import antenv

        mod = types.ModuleType("antenv.axon_hooks")
        mod._hook = None
        mod.set_axon_ntff_profile_hook = lambda h: setattr(mod, "_hook", h)
        mod.get_axon_ntff_profile_hook = lambda: mod._hook
        sys.modules["antenv.axon_hooks"] = mod
        antenv.axon_hooks = mod
        try:
            from trn_agent_boot.trn_boot import _ntff_profile_via_ctypes

            hook = _ntff_profile_via_ctypes("/opt/axon/libaxon_pjrt.so")
            if hook is not None:
                mod._hook = hook
                ok = True
        except Exception as e:  # profiling is best-effort
            print(f"ntff profile hook install failed: {e}", file=sys.stderr)
    if ok:
        import concourse.bass_utils as _bu

        _bu.upload_artifacts = lambda tmpdir: tmpdir
    return ok


def _build_program(causal: bool, exp_scale: float, cfg: dict) -> bass.Bass:
    a8 = cfg["a8"]
    v2s = cfg["v2s"] and a8
    ones8 = cfg["ones8"] and causal
    o8 = cfg["o8"] and causal
    o8h = cfg["o8h"] and causal and not o8

    nc = bacc.Bacc(None)
    S = SEQ

    x_dt = FP8 if a8 else BF16
    wo_dt = FP8 if o8 else BF16
    ao_dt = FP8 if o8 else BF16

    xT_d = nc.dram_tensor("xT", [D_MODEL, S], x_dt, kind="ExternalInput")
    if v2s:
        rT_d = nc.dram_tensor("rT", [D_MODEL, S], FP8, kind="ExternalInput")
    wqT_d = nc.dram_tensor("wqT", [D_MODEL, R_LOCAL], FP8, kind="ExternalInput")
    wkT_d = nc.dram_tensor("wkT", [D_MODEL, R_LOCAL], FP8, kind="ExternalInput")
    wvT_d = nc.dram_tensor("wvT", [D_MODEL, R_LOCAL], FP8, kind="ExternalInput")
    woT_d = nc.dram_tensor("woT", [R_LOCAL, D_MODEL], wo_dt, kind="ExternalInput")
    if o8h:
        woT8_d = nc.dram_tensor(
            "woT8", [2 * D_HEAD, D_MODEL], FP8, kind="ExternalInput"
        )
    # cos rows 0:64, sin rows 64:128
    cs_d = nc.dram_tensor("cossinT", [128, S], BF16, kind="ExternalInput")
    # swapped: sin rows 0:64, cos rows 64:128 (keeps TensorTensor base partitions equal)
    sc_d = nc.dram_tensor("sincosT", [128, S], BF16, kind="ExternalInput")
    if causal:
        # 16 transposed diagonal 128x128 mask blocks, side by side
        maskd_d = nc.dram_tensor("maskd", [128, S], BF16, kind="ExternalInput")
    else:
        maskf_d = nc.dram_tensor("maskf", [S, S], BF16, kind="ExternalInput")
    out_d = nc.dram_tensor("out", [S, D_MODEL], BF16, kind="ExternalOutput")

    xT_v = xT_d[:].rearrange("(mo p) s -> p mo s", p=128)
    if v2s:
        rT_v = rT_d[:].rearrange("(mo p) s -> p mo s", p=128)
    wqT_v = wqT_d[:].rearrange("(mo p) r -> p mo r", p=128)
    wkT_v = wkT_d[:].rearrange("(mo p) r -> p mo r", p=128)
    wvT_v = wvT_d[:].rearrange("(mo p) r -> p mo r", p=128)
    woT_v = woT_d[:].rearrange("(h p) o -> p h o", p=128)
    if o8h:
        woT8_v = woT8_d[:].rearrange("(h p) o -> p h o", p=128)
    if not causal:
        maskf_v = maskf_d[:].rearrange("(ko p) q -> p ko q", p=128)

    with tile.TileContext(nc) as tc:
        with tc.tile_pool(name="pers", bufs=1) as pers:
            # ---- persistent SBUF tensors (live across both phases) ----
            q_rot = pers.tile([128, HPC, S], BF16, tag="qrot")
            k_rot = pers.tile([128, HPC, S], BF16, tag="krot")
            v_sb = pers.tile([128, NKI, R_LOCAL], BF16, tag="vsb")
            aoT = pers.tile([128, HPC, S], ao_dt, tag="aoT")
            wo = pers.tile([128, HPC, D_MODEL], wo_dt, tag="wo")
            if o8h:
                aoT8 = pers.tile([128, 2, S], FP8, tag="aoT8")
                wo8 = pers.tile([128, 2, D_MODEL], FP8, tag="wo8")
            ones_t = pers.tile([128, 128], BF16, tag="ones")
            if ones8:
                ones2 = pers.tile([128, 2, 128], FP8, tag="ones2")
                nc.vector.memset(ones2[:], 1.0)
            warm = pers.tile([128, 2], BF16, tag="warm")
            if causal:
                maskd = pers.tile([128, S], BF16, tag="maskd")
            nc.vector.memset(ones_t[:, :], 1.0)
            # load the exp table set first so no ACT table switch happens
            # mid-kernel (Copy lives in every set).
            nc.vector.memset(warm[:, 0:1], 1.0)
            nc.scalar.activation(
                warm[:, 1:2], warm[:, 0:1], mybir.ActivationFunctionType.Exp
            )

            # ================= phase A: QKV projection + RoPE =================
            with (
                tc.tile_pool(name="xtp", bufs=1) as xtp,
                tc.tile_pool(name="wp", bufs=1) as wp,
                tc.tile_pool(name="raw", bufs=2) as rawp,
                tc.tile_pool(name="w8", bufs=6) as w8p,
                tc.tile_pool(name="tmp", bufs=2) as tmpp,
                tc.tile_pool(name="psA", bufs=4, space="PSUM") as psA,
            ):
                w_dt = FP8 if a8 else BF16
                xt = xtp.tile([128, MO, S], x_dt, tag="xt")
                if v2s:
                    rt = xtp.tile([128, MO, S], FP8, tag="rt")
                wq = wp.tile([128, MO, R_LOCAL], w_dt, tag="wq")
                wk = wp.tile([128, MO, R_LOCAL], w_dt, tag="wk")
                wv = wp.tile([128, MO, R_LOCAL], w_dt, tag="wv")
                cs_t = wp.tile([128, S], BF16, tag="cs")
                sc_t = wp.tile([128, S], BF16, tag="sc")

                def load_w(dst, view, mo4):
                    """load weight m-blocks mo4..mo4+3 (fp8 direct or cast)."""
                    if a8:  # straight fp8 DMA, no cast
                        nc.sync.dma_start(
                            out=dst[:, mo4 : mo4 + 4, :], in_=view[:, mo4 : mo4 + 4, :]
                        )
                    else:
                        for mo in range(mo4, mo4 + 4):
                            st = w8p.tile([128, R_LOCAL], FP8, tag="w8")
                            nc.sync.dma_start(out=st[:, :], in_=view[:, mo, :])
                            nc.vector.tensor_copy(dst[:, mo, :], st[:, :])

                # critical path first: wq chunk 0 + first x pairs feed the
                # h=0 q projection; w chunks and rope tables land ahead of
                # the bulk x blocks they gate.
                # each dma_start costs ~0.64us of serialized issue time on the
                # sync engine, so phase A uses few, chunky transfers; the DMA
                # queue is FIFO on transfers, so weight chunks interleave with
                # the x bulk (front-loading x delays the wk chunks the h=0
                # k-projection needs and measures slower).
                load_w(wq, wqT_v, 0)
                nc.sync.dma_start(out=xt[:, 0:2, 0:1024], in_=xT_v[:, 0:2, 0:1024])
                load_w(wk, wkT_v, 0)
                nc.sync.dma_start(
                    out=xt[:, 0:2, 1024:2048], in_=xT_v[:, 0:2, 1024:2048]
                )
                nc.sync.dma_start(out=xt[:, 2:4, :], in_=xT_v[:, 2:4, :])
                for c4m in range(1, 4):
                    load_w(wq, wqT_v, 4 * c4m)
                    load_w(wk, wkT_v, 4 * c4m)
                    nc.sync.dma_start(
                        out=xt[:, 4 * c4m : 4 * c4m + 4, :],
                        in_=xT_v[:, 4 * c4m : 4 * c4m + 4, :],
                    )
                # rope tables ride behind the x bulk: rope runs on the
                # (slack) vector engine, while a table transfer queued
                # mid-x stalls the tensor engine's m-pair consumption
                nc.sync.dma_start(out=cs_t[:, :], in_=cs_d[:, :])
                nc.sync.dma_start(out=sc_t[:, :], in_=sc_d[:, :])
                for c4m in range(4):
                    load_w(wv, wvT_v, 4 * c4m)
                    if v2s:
                        nc.sync.dma_start(
                            out=rt[:, 4 * c4m : 4 * c4m + 4, :],
                            in_=rT_v[:, 4 * c4m : 4 * c4m + 4, :],
                        )
                if causal:  # first needed in phase B
                    nc.sync.dma_start(out=maskd[:, :], in_=maskd_d[:, :])
                # prefetch the out-projection weights behind everything else
                # so phase B starts without a DMA wait
                if o8h:
                    nc.sync.dma_start(out=wo8[:, :, :], in_=woT8_v[:, :, :])
                    nc.sync.dma_start(out=wo[:, 2:4, :], in_=woT_v[:, 2:4, :])
                else:
                    for oc in range(D_MODEL // 512):
                        nc.sync.dma_start(
                            out=wo[:, :, oc * 512 : (oc + 1) * 512],
                            in_=woT_v[:, :, oc * 512 : (oc + 1) * 512],
                        )

                def rope(h, raw, dst):
                    """NeoX rotary: low = t1*c - t2*s ; hi = t1*s + t2*c."""
                    d_lo = dst[0:64, h, :]
                    d_hi = dst[64:128, h, :]
                    if cfg["rope4"]:
                        # 4 full-width ops, cross-base-partition sub/add
                        p1 = tmpp.tile([128, S], BF16, tag="tmp")
                        p2 = tmpp.tile([128, S], BF16, tag="tmp")
                        nc.vector.tensor_mul(p1[:, :], raw[:, :], cs_t[:, :])
                        nc.vector.tensor_mul(p2[:, :], raw[:, :], sc_t[:, :])
                        nc.vector.tensor_sub(d_lo, p1[0:64, :], p1[64:128, :])
                        nc.vector.tensor_add(d_hi, p2[0:64, :], p2[64:128, :])
                    else:
                        ta = tmpp.tile([64, S], BF16, tag="tmp")
                        tb = tmpp.tile([64, S], BF16, tag="tmp")
                        nc.vector.tensor_mul(ta[:, :], raw[0:64, :], cs_t[0:64, :])
                        nc.vector.tensor_mul(tb[:, :], raw[64:128, :], cs_t[64:128, :])
                        nc.vector.tensor_sub(d_lo, ta[:, :], tb[:, :])
                        tc2 = tmpp.tile([64, S], BF16, tag="tmp")
                        td = tmpp.tile([64, S], BF16, tag="tmp")
                        nc.vector.tensor_mul(tc2[:, :], raw[0:64, :], sc_t[0:64, :])
                        nc.vector.tensor_mul(td[:, :], raw[64:128, :], sc_t[64:128, :])
                        nc.vector.tensor_add(d_hi, tc2[:, :], td[:, :])

                def project_qk(h):
                    # two half-width psum tiles per projection: finer WAR
                    # rotation lets the next head's matmuls start while this
                    # head's second half is still being evicted
                    psq0 = psA.tile([128, 1024], F32, tag="psA")
                    psq1 = psA.tile([128, 1024], F32, tag="psA")
                    psk0 = psA.tile([128, 1024], F32, tag="psA")
                    psk1 = psA.tile([128, 1024], F32, tag="psA")
                    hs = slice(h * 128, (h + 1) * 128)

                    def mm_into(halves, w_sb, m_sl, first, last, pm):
                        for c4 in range(4):
                            cs_ = slice(c4 * 512, (c4 + 1) * 512)
                            tgt = halves[c4 // 2][:, (c4 % 2) * 512 : (c4 % 2) * 512 + 512]
                            nc.tensor.matmul(
                                tgt, w_sb[:, m_sl, hs], xt[:, m_sl, cs_],
                                start=first, stop=last, perf_mode=pm,
                            )

                    if a8:
                        for m2 in range(MO // 2):
                            ms = slice(2 * m2, 2 * m2 + 2)
                            mm_into((psq0, psq1), wq, ms, m2 == 0, m2 == MO // 2 - 1, DR)
                            mm_into((psk0, psk1), wk, ms, m2 == 0, m2 == MO // 2 - 1, DR)
                    else:
                        for m in range(MO):
                            mm_into((psq0, psq1), wq, m, m == 0, m == MO - 1, None)
                            mm_into((psk0, psk1), wk, m, m == 0, m == MO - 1, None)
                    q_raw = rawp.tile([128, S], BF16, tag="raw")
                    nc.scalar.copy(q_raw[:, 0:1024], psq0[:, :])
                    nc.scalar.copy(q_raw[:, 1024:2048], psq1[:, :])
                    rope(h, q_raw, q_rot)
                    k_raw = rawp.tile([128, S], BF16, tag="raw")
                    nc.scalar.copy(k_raw[:, 0:1024], psk0[:, :])
                    nc.scalar.copy(k_raw[:, 1024:2048], psk1[:, :])
                    rope(h, k_raw, k_rot)

                for h in range(HPC):
                    project_qk(h)

                # ---- V projection (natural layout [s, r]) ----
                for sb2 in range(NSB // 2):
                    ps = psA.tile([128, 1024], F32, tag="psA")
                    for part in range(2):
                        sb = sb2 * 2 + part
                        ss = slice(sb * 128, (sb + 1) * 128)
                        ps_ = ps[:, part * 512 : part * 512 + 512]
                        if a8:
                            for m2 in range(MO // 2):
                                ms = slice(2 * m2, 2 * m2 + 2)
                                nc.tensor.matmul(
                                    ps_, xt[:, ms, ss], wv[:, ms, :],
                                    start=(m2 == 0),
                                    stop=(not v2s and m2 == MO // 2 - 1),
                                    perf_mode=DR,
                                )
                            if v2s:  # residual stream restores bf16 accuracy
                                for m2 in range(MO // 2):
                                    ms = slice(2 * m2, 2 * m2 + 2)
                                    nc.tensor.matmul(
                                        ps_, rt[:, ms, ss], wv[:, ms, :],
                                        start=False, stop=(m2 == MO // 2 - 1),
                                        perf_mode=DR,
                                    )
                        else:
                            for m in range(MO):
                                nc.tensor.matmul(
                                    ps_, xt[:, m, ss], wv[:, m, :],
                                    start=(m == 0), stop=(m == MO - 1),
                                )
                    dst = v_sb[:, sb2 * 2 : sb2 * 2 + 2, :]
                    nc.scalar.copy(dst, ps[:, :])

            # ================= phase B: attention + out-projection =============
            with (
                tc.tile_pool(name="pp", bufs=8) as ppp,
                tc.tile_pool(name="pp8", bufs=8) as pp8p,
                tc.tile_pool(name="rcp", bufs=3) as rcp,
                tc.tile_pool(name="osb", bufs=4) as osbp,
                tc.tile_pool(name="mblk", bufs=4) as mblkp,
                tc.tile_pool(name="sp", bufs=4, space="PSUM") as spp,
                tc.tile_pool(name="acc", bufs=2, space="PSUM") as accp,
            ):
                evict_flip = [0]
                EXPF = mybir.ActivationFunctionType.Exp

                def attn_causal(qc, h, sums, avp):
                    """per-ki pipeline; full-block denominators collected as
                    fp8 pairs and summed by deferred DoubleRow matmuls."""
                    q_lo = qc * 512
                    hs = slice(h * 128, (h + 1) * 128)
                    nki_here = 4 * qc + 4
                    pp8s = []
                    for ki in range(nki_here):
                        diag = ki >= 4 * qc
                        q0 = 128 * (ki - 4 * qc) if diag else 0
                        spbv = spp.tile([128, 512], F32, tag="sp")
                        pp = ppp.tile([128, 512], BF16, tag="pp")
                        nc.tensor.matmul(
                            spbv[:, q0:512],
                            k_rot[:, h, ki * 128 : (ki + 1) * 128],
                            q_rot[:, h, q_lo + q0 : q_lo + 512],
                            start=True, stop=True,
                        )
                        nc.scalar.activation(
                            pp[:, q0:512], spbv[:, q0:512], EXPF,
                            scale=float(exp_scale),
                        )
                        if diag:
                            nc.vector.tensor_mul(
                                pp[:, q0 : q0 + 128], pp[:, q0 : q0 + 128],
                                maskd[:, ki * 128 : (ki + 1) * 128],
                            )
                            # diag blocks feed the bf16 denominator directly;
                            # under ones8 the first diag (covering [0:512])
                            # opens the accumulation group
                            nc.tensor.matmul(
                                sums[:, q0:512], ones_t[:, :], pp[:, q0:512],
                                start=(ki == (4 * qc if ones8 else 0)),
                                stop=(ki == nki_here - 1 and not (ones8 and qc)),
                            )
                        elif ones8:
                            if ki % 2 == 0:
                                pp8 = pp8p.tile([128, 2, 512], FP8, tag="pp8")
                                pp8s.append(pp8)
                            nc.vector.tensor_copy(pp8s[-1][:, ki % 2, :], pp[:, :])
                        else:
                            nc.tensor.matmul(
                                sums[:, :], ones_t[:, :], pp[:, :],
                                start=(ki == 0), stop=False,
                            )
                        nc.tensor.matmul(
                            avp[:, q0:512], v_sb[:, ki, hs], pp[:, q0:512],
                            start=(ki == 0), stop=(ki == nki_here - 1),
                        )
                    # deferred fp8 DoubleRow denominator over full-ki pairs:
                    # their vector copies completed long ago, so the in-order
                    # PE reaches these with no wait
                    for p, pp8 in enumerate(pp8s):
                        nc.tensor.matmul(
                            sums, ones2[:, :, :], pp8[:, :, :],
                            perf_mode=DR, start=False, stop=(p == len(pp8s) - 1),
                        )

                def attn_full(qc, h, sums, avp):
                    """non-causal fallback: every ki block, host-supplied mask."""
                    q_lo = qc * 512
                    hs = slice(h * 128, (h + 1) * 128)
                    for ki in range(NKI):
                        spbv = spp.tile([128, 512], F32, tag="sp")
                        pp = ppp.tile([128, 512], BF16, tag="ppf")
                        nc.tensor.matmul(
                            spbv[:, :],
                            k_rot[:, h, ki * 128 : (ki + 1) * 128],
                            q_rot[:, h, q_lo : q_lo + 512],
                            start=True, stop=True,
                        )
                        nc.scalar.activation(
                            pp[:, :], spbv[:, :], EXPF, scale=float(exp_scale)
                        )
                        mb = mblkp.tile([128, 512], BF16, tag="mblk")
                        nc.sync.dma_start(
                            out=mb[:, :], in_=maskf_v[:, ki, q_lo : q_lo + 512]
                        )
                        nc.vector.tensor_mul(pp[:, 0:512], pp[:, 0:512], mb[:, :])
                        nc.tensor.matmul(
                            sums[:, :], ones_t[:, :], pp[:, :],
                            start=(ki == 0), stop=(ki == NKI - 1),
                        )
                        nc.tensor.matmul(
                            avp[:, :], v_sb[:, ki, hs], pp[:, :],
                            start=(ki == 0), stop=(ki == NKI - 1),
                        )

                for qc in range(NQC):
                    q_lo = qc * 512
                    for h in range(HPC):
                        sav = accp.tile([128, 2, 512], F32, tag="acc")
                        sums = sav[:, 0, :]
                        avp = sav[:, 1, :]
                        if causal:
                            attn_causal(qc, h, sums, avp)
                        else:
                            attn_full(qc, h, sums, avp)
                        rc = rcp.tile([128, 512], F32, tag="rc")
                        nc.vector.reciprocal_approx_fast(rc[:, :], sums[:, :])
                        if o8h and h < 2:
                            ao_dst = aoT8[:, h, q_lo : q_lo + 512]
                        else:
                            ao_dst = aoT[:, h, q_lo : q_lo + 512]
                        nc.vector.tensor_mul(ao_dst, avp[:, :], rc[:, :])

                    # out-projection for this query chunk (4 seq blocks);
                    # both 1024-wide halves land in one tile -> one DMA per sb
                    for sb in range(4 * qc, 4 * qc + 4):
                        ss = slice(sb * 128, (sb + 1) * 128)
                        ob = osbp.tile([128, 2048], BF16, tag="osb")
                        for oc2 in range(2):
                            op2 = accp.tile([128, 2, 512], F32, tag="acc")
                            if o8:
                                for hp in range(2):
                                    for ocp in range(2):
                                        oc = 2 * oc2 + ocp
                                        os_ = slice(oc * 512, (oc + 1) * 512)
                                        nc.tensor.matmul(
                                            op2[:, ocp, :],
                                            aoT[:, 2 * hp : 2 * hp + 2, ss],
                                            wo[:, 2 * hp : 2 * hp + 2, os_],
                                            start=(hp == 0), stop=(hp == 1),
                                            perf_mode=DR,
                                        )
                            elif o8h:
                                # heads 0-1 as one fp8 DoubleRow pair,
                                # heads 2-3 bf16
                                for ocp in range(2):
                                    oc = 2 * oc2 + ocp
                                    os_ = slice(oc * 512, (oc + 1) * 512)
                                    nc.tensor.matmul(
                                        op2[:, ocp, :],
                                        aoT8[:, :, ss], wo8[:, :, os_],
                                        start=True, stop=False, perf_mode=DR,
                                    )
                                for hh in range(2, HPC):
                                    for ocp in range(2):
                                        oc = 2 * oc2 + ocp
                                        os_ = slice(oc * 512, (oc + 1) * 512)
                                        nc.tensor.matmul(
                                            op2[:, ocp, :],
                                            aoT[:, hh, ss], wo[:, hh, os_],
                                            start=False, stop=(hh == HPC - 1),
                                        )
                            else:
                                for hh in range(HPC):
                                    for ocp in range(2):
                                        oc = 2 * oc2 + ocp
                                        os_ = slice(oc * 512, (oc + 1) * 512)
                                        nc.tensor.matmul(
                                            op2[:, ocp, :],
                                            aoT[:, hh, ss], wo[:, hh, os_],
                                            start=(hh == 0), stop=(hh == HPC - 1),
                                        )
                            obh = ob[:, oc2 * 1024 : (oc2 + 1) * 1024]
                            if evict_flip[0] % 2 == 0:
                                nc.scalar.copy(obh, op2[:, :, :])
                            else:
                                nc.vector.tensor_copy(obh, op2[:, :, :])
                            evict_flip[0] += 1
                        nc.sync.dma_start(out=out_d[ss, :], in_=ob[:, :])

    nc.finalize()
    return nc


def _bit_quantize_ternary(w: np.ndarray):
    """Returns (ternary {-1,0,1} float32 matrix, scale) matching the reference."""
    scale = np.maximum(np.mean(np.abs(w.astype(np.float32))), np.float32(1e-5))
    t = np.clip(np.round(w.astype(np.float32) / scale), -1.0, 1.0).astype(np.float32)
    return t, float(scale)


def _host_tables():
    """cos/sin stacked [128, S]: rows 0:64 cos, rows 64:128 sin."""
    inv_freq = 1.0 / (ROPE_BASE ** (np.arange(0, D_HEAD, 2, dtype=np.float32) / D_HEAD))
    pos = np.arange(SEQ, dtype=np.float32)
    ang = pos[:, None] * inv_freq[None, :]  # [S, 64]
    cs = np.empty((128, SEQ), dtype=NPBF16)
    cs[0:64] = np.ascontiguousarray(np.cos(ang).T).astype(NPBF16)
    cs[64:128] = np.ascontiguousarray(np.sin(ang).T).astype(NPBF16)
    sc = np.empty((128, SEQ), dtype=NPBF16)
    sc[0:64] = cs[64:128]
    sc[64:128] = cs[0:64]
    return cs, sc


def kernel(x, w_qkv, w_out, mask):
    global LAST_RESULT
    x = np.asarray(x, dtype=np.float32)
    w_qkv = np.asarray(w_qkv, dtype=np.float32)
    w_out = np.asarray(w_out, dtype=np.float32)
    mask = np.asarray(mask)

    tq, sq = _bit_quantize_ternary(w_qkv)
    to, so = _bit_quantize_ternary(w_out)
    exp_scale = (sq * sq) / float(np.sqrt(D_HEAD))
    c2 = np.float32(sq * so)

    m2 = (mask.reshape(SEQ, SEQ) != 0).astype(np.float32)
    causal = bool(np.array_equal(m2, np.tril(np.ones((SEQ, SEQ), np.float32))))

    cfg = dict(CFG)
    o8 = cfg["o8"] and causal
    o8h = cfg["o8h"] and causal and not o8

    cs, sc = _host_tables()
    if causal:
        maskd = np.empty((128, SEQ), dtype=NPBF16)
        for ki in range(NKI):
            blk = m2[ki * 128 : (ki + 1) * 128, ki * 128 : (ki + 1) * 128]  # [q, k]
            maskd[:, ki * 128 : (ki + 1) * 128] = np.ascontiguousarray(blk.T).astype(
                NPBF16
            )
    else:
        maskf = np.ascontiguousarray(m2.T).astype(NPBF16)  # [kk, qq]

    key = (causal, float(exp_scale), tuple(sorted(cfg.items())))
    if key not in _PROG_CACHE:
        _PROG_CACHE[key] = _build_program(causal, float(exp_scale), cfg)
    nc = _PROG_CACHE[key]

    np_x = NPFP8 if cfg["a8"] else NPBF16
    np_wo = NPFP8 if o8 else NPBF16
    v2s = cfg["v2s"] and cfg["a8"]
    xTs, rTs = [], []
    for b in range(BATCH):
        xT = np.ascontiguousarray(x[b].T)
        x8 = xT.astype(np_x)
        xTs.append(x8)
        if v2s:
            rTs.append((xT - x8.astype(np.float32)).astype(NPFP8))
    in_maps = []
    for c in range(N_CORES):
        b, g = divmod(c, 4)
        rows = slice(R_LOCAL * g, R_LOCAL * (g + 1))
        woT_np = np.ascontiguousarray(to[:, rows].T)
        im = {
            "xT": xTs[b],
            "wqT": np.ascontiguousarray(tq[0 * D_MODEL :][rows].T).astype(NPFP8),
            "wkT": np.ascontiguousarray(tq[1 * D_MODEL :][rows].T).astype(NPFP8),
            "wvT": np.ascontiguousarray(tq[2 * D_MODEL :][rows].T).astype(NPFP8),
            "woT": woT_np.astype(np_wo),
            "cossinT": cs,
            "sincosT": sc,
        }
        if o8h:
            im["woT8"] = np.ascontiguousarray(woT_np[0 : 2 * D_HEAD]).astype(NPFP8)
        if v2s:
            im["rT"] = rTs[b]
        if causal:
            im["maskd"] = maskd
        else:
            im["maskf"] = maskf
        in_maps.append(im)

    do_trace = bool(PROFILE) and _enable_profiling()
    res = run_bass_kernel_spmd(nc, in_maps, list(range(N_CORES)), trace=do_trace)
    LAST_RESULT = res

    parts = [np.asarray(res.results[c]["out"]).astype(np.float32) for c in range(N_CORES)]
    out = np.stack(
        [
            parts[0] + parts[1] + parts[2] + parts[3],
            parts[4] + parts[5] + parts[6] + parts[7],
        ]
    )
    return (out * c2).astype(np.float32)
